# revision 9
# baseline (speedup 1.0000x reference)
"""Trainium2 Bass kernel for nn_ConvMatrix2d (CapsNet-style matrix-capsule conv, k=1, s=1).

Computation (per batch b, input-capsule c, spatial position ji = J*14+I):
    out[b, c, o*196 + ji, p*4+r] = sum_q W[c,o,p,q] * x[b,c,I,J,q*4+r]
    out[b, c, o*196 + ji, 16]    = x[b,c,I,J,16]
Output: (32, 32, 6272, 17) fp32 = 437 MB  -> heavily output-DMA bound.

Strategy (8 cores, data parallel over batch: 4 batches/core):
  - Votes for 4 channels (c_lo) at once via ONE block-diagonal matmul:
    lhsT[(c_lo,q), (c_lo',o)] = W[c,o,p,q] iff c_lo==c_lo' (16x128, zeros
    elsewhere), rhs[(c_lo,q), (ji,r)] = pose rows of the 4 channels. That
    fills all 128 output partitions (c_lo,o) from a single rhs stream, so
    the PE streams the minimum number of moving columns.
  - float32r matmuls (1 cycle/col at N>=392 vs 4 for fp32) -> tensor
    engine drops from ~416us to <100us per core; output DMA (54.7 MB @
    ~358 GB/s) becomes the bottleneck.
  - Acts broadcast over o via a tiny K=4 delta-matmul from partition
    strip 1 (rows 32..36).
  - DVE/ACT interleave-copy PSUM -> staging tile [128=(c_lo,o), 3332=
    (ji,t)] which is exactly HBM layout; one 1.7MB out-DMA per (b, c_hi)
    with 13.3KB-contiguous descriptors across all 128 partitions.
"""

import numpy as np

import concourse.bass as bass
import concourse.bacc as bacc
import concourse.mybir as mybir
from concourse.tile import TileContext
from concourse.bass_utils import run_bass_kernel_spmd

# Problem constants (hardcoded per contract)
B, C, WSP, HH = 32, 32, 14, 17
O, H = 32, 4
JI = WSP * WSP          # 196
NB = 4                  # batches per core
NCORES = 8
CHI, CLO = 8, 4         # c = c_hi*4 + c_lo
HJ = 98                 # ji per half
FH = HJ * 4             # 392 moving cols per half (ji x r)
ROW = HH                # 17 floats per output row
SLAB = JI * HH          # 3332 floats per (b,c,o)

F32 = mybir.dt.float32
F32R = mybir.dt.float32r


def _build_nc():
    nc = bacc.Bacc()
    x_d = nc.dram_tensor("x3", [NB, CHI, 16, 784], F32R, kind="ExternalInput")
    xa_d = nc.dram_tensor("xa", [NB, CHI, 4, 196], F32R, kind="ExternalInput")
    w_d = nc.dram_tensor("w3", [16, CHI * 4 * 128], F32R, kind="ExternalInput")
    wa_d = nc.dram_tensor("wact", [4, 128], F32R, kind="ExternalInput")
    out_d = nc.dram_tensor("out", [NB, C, O * JI, HH], F32, kind="ExternalOutput")

    with TileContext(nc) as tc:
        with (
            tc.tile_pool(name="wpool", bufs=1) as wpool,
            tc.tile_pool(name="xpool", bufs=4) as xpool,
            tc.tile_pool(name="stage", bufs=6) as spool,
            tc.tile_pool(name="psv", bufs=6, space="PSUM") as pv_pool,
            tc.tile_pool(name="psa", bufs=2, space="PSUM") as pa_pool,
        ):
            # Resident weights.
            # w_sb rows 0..16 = (c_lo, q); free = (c_hi, p, 128 block-diag cols)
            wact_sb = wpool.tile([128, 128], F32R)
            nc.sync.dma_start(out=wact_sb[32:36, :], in_=wa_d[:, :])
            w_sb = wpool.tile([16, CHI * 4 * 128], F32R)
            nc.sync.dma_start(out=w_sb[:, 0:512], in_=w_d[:, 0:512])
            nc.sync.dma_start(out=w_sb[:, 512:], in_=w_d[:, 512:])

            HSLAB = HJ * ROW    # 1666 floats per half-slab row
            iters = [(b, c_hi) for b in range(NB) for c_hi in range(CHI)]
            PF = 2              # x prefetch depth (iterations ahead)
            x_tiles = {}

            def load_x(k):
                bb, ch_ = iters[k]
                t = xpool.tile([128, 784], F32R, tag="x")
                # pose rows (c_lo,q) at partitions 0..16; act rows at 32..36.
                # Issued on the sync ring BEFORE this iteration's out-DMAs
                # (software-pipelined prefetch; never queues behind a
                # blocking stage-ready wait).
                nc.sync.dma_start(out=t[0:16, :], in_=x_d[bb, ch_])
                nc.sync.dma_start(out=t[32:36, 0:196], in_=xa_d[bb, ch_])
                x_tiles[k] = t

            for k in range(PF):
                load_x(k)
            for k, (b, c_hi) in enumerate(iters):
                    if k + PF < len(iters):
                        load_x(k + PF)
                    x_sb = x_tiles.pop(k)

                    for half in range(2):
                        cl, ch = half * FH, (half + 1) * FH
                        stage = spool.tile([128, HSLAB], F32, tag="stage")
                        sview = stage.rearrange("z (ji t) -> z ji t", t=ROW)

                        at = pa_pool.tile([128, HJ], F32, tag="a")
                        nc.tensor.matmul(
                            at[:, :],
                            wact_sb[32:36, :],
                            x_sb[32:36, half * HJ:(half + 1) * HJ],
                        )
                        nc.vector.tensor_copy(sview[:, :, 16], at[:, :])

                        for p in range(4):
                            vt = pv_pool.tile([128, FH], F32, tag="v")
                            lhsT = w_sb[0:16,
                                        (c_hi * 4 + p) * 128:(c_hi * 4 + p + 1) * 128]
                            nc.tensor.matmul(
                                vt[:, :],
                                lhsT,
                                x_sb[0:16, cl:ch],
                            )
                            # Interleave-copy PSUM -> staging rows (ji*17 + p*4 + r)
                            src = vt.rearrange("z (jj r) -> z jj r", r=4)
                            dst = sview[:, :, p * 4: p * 4 + 4]
                            if p < 2:
                                nc.vector.tensor_copy(dst, src)
                            else:
                                nc.scalar.copy(dst, src)

                        # 0.85MB half out-DMA: dst [c_lo 4][o 32][1666 contig]
                        dst = out_d.rearrange(
                            "b (ch cl) (o j) t -> b ch cl o (j t)", cl=CLO, o=O
                        )[b, c_hi, :, :, half * HSLAB:(half + 1) * HSLAB]
                        nc.sync.dma_start(out=dst, in_=stage[:])
    if not nc.is_finalized():
        nc.finalize()
    return nc


_CACHE = {}


def _get_nc():
    if "nc" not in _CACHE:
        _CACHE["nc"] = _build_nc()
    return _CACHE["nc"]


def _preprocess(x, weight):
    """Build per-core input maps from full inputs."""
    x = np.ascontiguousarray(x, dtype=np.float32)
    weight = np.ascontiguousarray(weight, dtype=np.float32)
    xp = x.transpose(0, 1, 3, 2, 4).reshape(B, C, JI, HH)  # ji = J*14+I
    # x3[b, c_hi, (c_lo,q), (ji,r)] pose rows; xa[b, c_hi, c_lo, ji] acts
    pose = xp[..., :16].reshape(B, C, JI, 4, 4)            # [b,c,ji,q,r]
    x3p = np.ascontiguousarray(
        pose.transpose(0, 1, 3, 2, 4).reshape(B, CHI, 16, 784))
    xa = np.ascontiguousarray(xp[..., 16].reshape(B, CHI, 4, 196))

    Wm = weight[:, 0, 0]                                   # (C, O, 4, 4): W[c,o,p,q]
    w3 = np.zeros((16, CHI, 4, 128), dtype=np.float32)
    WmB = Wm.reshape(CHI, CLO, O, 4, 4)                    # [c_hi, c_lo, o, p, q]
    for c_lo in range(CLO):
        # dst w3[c_lo*4+q, c_hi, p, c_lo*32+o]
        w3[c_lo * 4:c_lo * 4 + 4, :, :, c_lo * 32:c_lo * 32 + 32] = (
            WmB[:, c_lo].transpose(3, 0, 2, 1))            # (q, c_hi, p, o)
    w3 = np.ascontiguousarray(w3.reshape(16, CHI * 4 * 128))

    wact = np.zeros((4, 128), dtype=np.float32)
    for c_lo in range(CLO):
        wact[c_lo, c_lo * 32:(c_lo + 1) * 32] = 1.0

    in_maps = []
    for k in range(NCORES):
        in_maps.append({
            "x3": np.ascontiguousarray(x3p[k * NB:(k + 1) * NB]),
            "xa": np.ascontiguousarray(xa[k * NB:(k + 1) * NB]),
            "w3": w3,
            "wact": wact,
        })
    return in_maps


def _run(x, weight, trace=False, trace_kwargs=None):
    nc = _get_nc()
    in_maps = _preprocess(x, weight)
    res = run_bass_kernel_spmd(
        nc, in_maps, list(range(NCORES)), trace=trace,
        trace_kwargs=trace_kwargs or {},
    )
    out = np.concatenate([r["out"] for r in res.results], axis=0)
    return out.astype(np.float32, copy=False), res


def kernel(x, weight):
    out, _ = _run(x, weight)
    return out
